# revision 1
# baseline (speedup 1.0000x reference)
"""Trainium2 Bass kernel for nn_MultiHeadAttention_34162169872901.

MultiHeadAttention (B=4, S=2048, d_model=512, 8 heads, d_k=64) with a
relative-position bias table (511 entries, clamp +-255) and an all-ones mask.

Sharding (8 NeuronCores): core c handles batch b = c//2 and 4 of the 8 heads
(c%2 selects the head half) -- data parallel on B, tensor parallel on heads.
Each core computes its 4 heads' Q/K/V projections, the full attention for its
batch, normalization, and its partial output projection; the host sums the two
partial outputs per batch (and adds the output bias bo).

On-device layout / algorithm per core:
  - Host pre-transposes activations to [d_model, S] so the contraction dim is
    on partitions, and pre-arranges weight slices; the 1/sqrt(d_k) scale is
    folded into Wq.
  - Projections produce QT/KT in [head-pair(128), S] layout and V in natural
    [S, d] layout (with a ones column per head for the softmax denominator).
  - Scores are computed transposed (S^T[k, q], k on partitions) so the
    relative-position bias becomes per-(k-tile, q-chunk) Toeplitz blocks;
    blocks fully outside the clamp band are constants folded into the exp's
    per-partition bias; in-band blocks multiply exp(scores) by precomputed
    exp(bias) blocks in bf16 on the vector engine.
  - AV matmul uses V (plus a ones column) as the stationary operand, giving
    ctx^T[d, q] and the softmax denominator l[q] in one accumulation chain.
  - Normalization: approximate reciprocal of l, broadcast to 64 rows via a
    tiny matmul, fused multiply into the O-projection stationary.
  - O-projection accumulates all 4 local heads into [q, 512] PSUM tiles.
"""

import sys
import types

import numpy as np

B = 4
S = 2048
D = 512
NHEAD = 8
DK = 64
NCORES = 8
MAX_REL = 255
NKT = S // 128   # 16 k-tiles
NU = S // 512    # 4 q-units


def _install_axon_hooks():
    """Provide antenv.axon_hooks (missing in this image) so bass_utils'
    trace path can be used; harmless when tracing is off."""
    try:
        import antenv
    except ImportError:
        return
    try:
        from antenv.axon_hooks import get_axon_ntff_profile_hook  # noqa: F401
        return
    except ImportError:
        pass
    hook = None
    try:
        from trn_agent_boot.trn_boot import _ntff_profile_via_ctypes
        hook = _ntff_profile_via_ctypes("/opt/axon/libaxon_pjrt.so")
    except Exception:
        hook = None
    m = types.ModuleType("antenv.axon_hooks")
    m.get_axon_ntff_profile_hook = lambda: hook
    m.set_axon_ntff_profile_hook = lambda h: None
    sys.modules["antenv.axon_hooks"] = m
    antenv.axon_hooks = m


_install_axon_hooks()

import concourse.bass as bass  # noqa: E402
import concourse.bacc as bacc  # noqa: E402
import concourse.mybir as mybir  # noqa: E402
from concourse import tile  # noqa: E402
from concourse.bass_utils import run_bass_kernel_spmd  # noqa: E402
from concourse.vector_clock import ScopedClock as _ScopedClock  # noqa: E402

f32 = mybir.dt.float32
f32r = mybir.dt.float32r
bf16 = mybir.dt.bfloat16
f16 = mybir.dt.float16
AF = mybir.ActivationFunctionType


def _patched_drain_and_barrier(self, tick_clock, wait_clock):
    # walrus in this container rejects >2 sem waits on one instruction; emit
    # the tail-drain waits as standalone wait instructions instead.
    nc = self.nc
    dummy = mybir.InstNoOp(name="drain-wait-probe", engine=mybir.EngineType.SP)
    wait_clock.add_sem_waits(dummy, _ScopedClock({None: tick_clock.global_clock}))
    handles = {h.name: h for h in self.sems.allocated().values()}
    si = dummy.sync_info
    for w in (si.on_wait if si is not None else []):
        nc.sync.wait_ge(handles[w.ant_name], w.wait_value)
    nc.sync.drain()
    nc.all_engine_barrier()
    popped = nc._tile_sem_poison_stack.pop()
    assert popped is self._sem_poison
    nc.clear_and_free_semaphores(list(self.sems.allocated().values()))
    nc.all_engine_barrier()


tile.TileContext._drain_and_barrier = _patched_drain_and_barrier


def _delta(t, u):
    # key-tile offset minus query-chunk offset; bias entry index is
    # delta + (p - f) + 255 clipped to [0, 510]
    return 128 * t - 512 * u


def _cls(t, u):
    d = _delta(t, u)
    if d <= -384:
        return 1  # whole block clamps to table[0]
    if d >= 768:
        return 2  # whole block clamps to table[510]
    return 0      # in-band: needs the Toeplitz block


def _didx(t, u):
    return (_delta(t, u) + 256) // 128  # 0..7 for in-band blocks


def build_program():
    nc = bacc.Bacc()

    xqT = nc.declare_dram_parameter("xqT", [D, S], f16, isOutput=False)
    xkT = nc.declare_dram_parameter("xkT", [D, S], f16, isOutput=False)
    xvT = nc.declare_dram_parameter("xvT", [D, S], f16, isOutput=False)
    wq = nc.declare_dram_parameter("wq", [128, 4, 256], f16, isOutput=False)
    wk = nc.declare_dram_parameter("wk", [128, 4, 256], f16, isOutput=False)
    wv = nc.declare_dram_parameter("wv", [128, 4, 256], f16, isOutput=False)
    wo = nc.declare_dram_parameter("wo", [64, 4, 512], f32r, isOutput=False)
    ebd = nc.declare_dram_parameter("eb", [128, 4, 8, 512], f16, isOutput=False)
    cbd = nc.declare_dram_parameter("cb", [128, 4, 3], f32, isOutput=False)
    outd = nc.declare_dram_parameter("out", [S, D], f32, isOutput=True)

    with tile.TileContext(nc) as tc:
        with (
            tc.tile_pool(name="sb", bufs=1) as pool,
            tc.tile_pool(name="xt", bufs=2) as xpool,
            tc.tile_pool(name="pt", bufs=4) as ppool,
            tc.tile_pool(name="cxp", bufs=3) as cpool,
            tc.tile_pool(name="obp", bufs=4) as opool,
        ):
            # ---- persistent SBUF tiles -------------------------------------
            wq_sb = pool.tile([128, 4, 256], f16, tag="wq")
            wk_sb = pool.tile([128, 4, 256], f16, tag="wk")
            wv_sb = pool.tile([128, 4, 256], f16, tag="wv")
            wo_sb = pool.tile([64, 4, 512], f32r, tag="wo")
            eb_sb = pool.tile([128, 4, 8, 512], f16, tag="eb")
            cb_sb = pool.tile([128, 4, 3], f32, tag="cb")
            qt_sb = pool.tile([128, 2, S], f16, tag="qt")
            kt_sb = pool.tile([128, 2, S], f16, tag="kt")
            v_sb = pool.tile([128, NKT, 4 * 65], f16, tag="v")
            ones_c = pool.tile([1, 64], mybir.dt.float16, tag="ones")
            warm = pool.tile([128, 16], f32, tag="warm")

            nc.sync.dma_start(wq_sb[:], wq[:])
            nc.sync.dma_start(wk_sb[:], wk[:])
            nc.sync.dma_start(wv_sb[:], wv[:])
            nc.vector.memset(ones_c[:], 1.0)
            # preload the exp table while DMAs stream in
            nc.vector.memset(warm[:], 0.0)
            nc.scalar.activation(warm[:], warm[:], AF.Exp, bias=0.0, scale=1.0)

            # ---- phase A: projections --------------------------------------
            with tc.tile_pool(name="pa", bufs=8, space="PSUM") as pa:
                _dma_rest = True
                # Q and K -> [head-pair 128, S] (transposed) layout
                # xv streams on the gpsimd queue ahead of eb/cb/wo
                xts = []
                for ct in range(4):
                    xv_t = xpool.tile([128, S], f16, tag="xv", bufs=4, name=f"xv{ct}")
                    nc.gpsimd.dma_start(xv_t[:], xvT[ct * 128:(ct + 1) * 128, :])
                    xts.append(xv_t)
                nc.gpsimd.dma_start(eb_sb[:], ebd[:])
                nc.gpsimd.dma_start(cb_sb[:], cbd[:])
                nc.gpsimd.dma_start(wo_sb[:], wo[:])
                # K and Q hp0 stream per-arriving c-tile (8 banks); hp1 and V
                # run as dense backlog to keep the PE warm into phase B
                kts, qts = [], []
                for ct in range(4):
                    xk_t = xpool.tile([128, S], f16, tag="xt", name=f"xk{ct}")
                    nc.sync.dma_start(xk_t[:], xkT[ct * 128:(ct + 1) * 128, :])
                    kts.append(xk_t)
                    xq_t = xpool.tile([128, S], f16, tag="xq", bufs=4, name=f"xq{ct}")
                    nc.scalar.dma_start(xq_t[:], xqT[ct * 128:(ct + 1) * 128, :])
                    qts.append(xq_t)
                for hp in range(2):
                    pks = {sc: pa.tile([128, 512], f32, tag="pa", name=f"pk{hp}_{sc}")
                           for sc in range(4)}
                    pqs = {sc: pa.tile([128, 512], f32, tag="pa", name=f"pq{hp}_{sc}")
                           for sc in range(4)}
                    for ct in range(4):
                        for sc in range(4):
                            nc.tensor.matmul(
                                pks[sc][:],
                                lhsT=wk_sb[:, ct, hp * 128:(hp + 1) * 128],
                                rhs=kts[ct][:, sc * 512:(sc + 1) * 512],
                                start=(ct == 0), stop=(ct == 3),
                            )
                            nc.tensor.matmul(
                                pqs[sc][:],
                                lhsT=wq_sb[:, ct, hp * 128:(hp + 1) * 128],
                                rhs=qts[ct][:, sc * 512:(sc + 1) * 512],
                                start=(ct == 0), stop=(ct == 3),
                            )
                    for sc in range(4):
                        nc.vector.tensor_copy(
                            kt_sb[:, hp, sc * 512:(sc + 1) * 512], pks[sc][:])
                        nc.vector.tensor_copy(
                            qt_sb[:, hp, sc * 512:(sc + 1) * 512], pqs[sc][:])
                # V -> natural [s, d] layout in bf16, one 65-col group per head
                # one accumulation group open per bank at a time: run each
                # s-tile's 4-step contraction to completion (xv tiles resident)
                pv = [pa.tile([128, 512], f32, tag="pa", name=f"pv{i}") for i in range(8)]
                for st in range(NKT):
                    for ct in range(4):
                        nc.tensor.matmul(
                            pv[st // 2][:, (st % 2) * 256:(st % 2) * 256 + 256],
                            lhsT=xts[ct][:, st * 128:(st + 1) * 128],
                            rhs=wv_sb[:, ct, :],
                            start=(ct == 0), stop=(ct == 3),
                        )
                for st in range(NKT):
                    vslice = v_sb[:, st, :].rearrange("p (h x) -> p h x", x=65)
                    nc.vector.tensor_copy(
                        vslice[:, :, 0:64],
                        pv[st // 2][:, (st % 2) * 256:(st % 2) * 256 + 256]
                        .rearrange("p (h x) -> p h x", x=64),
                    )
                    nc.vector.memset(vslice[:, :, 64:65], 1.0)

            # ---- phase B: attention + normalization + O-projection --------
            import os as _os
            _phase = _os.environ.get("KPHASE", "full")
            if _phase == "A":
                ob0 = opool.tile([128, 512], f32, tag="ob", name="ob0")
                nc.vector.tensor_copy(ob0[:], qt_sb[:, 0, 0:512].bitcast(f32))
                nc.sync.dma_start(outd[0:128, :], ob0[:])
                ob1 = opool.tile([128, 256], f32, tag="ob1", name="ob1")
                nc.vector.tensor_copy(ob1[:], v_sb[:, 0, 0:256])
                nc.sync.dma_start(outd[128:256, 0:256], ob1[:])
            _enable_b = _phase != "A"
            with (
                tc.tile_pool(name="sc", bufs=3, space="PSUM") as scp,
                tc.tile_pool(name="c1", bufs=1, space="PSUM") as c1p,
            ):
                _lvl = int(_os.environ.get("KLEVEL", "5"))
                for u in (range(NU) if _enable_b else []):
                    cx = {}
                    for hp in range(2):
                        if _lvl >= 3:
                            ctxp = [c1p.tile([65, 512], f32, tag="cp", bufs=2, name=f"ctxp{i}") for i in range(2)]
                        nav = [0, 0]
                        gorder = sorted(range(NKT // 2),
                                        key=lambda g: (_cls(2 * g, u) == 0, g))
                        for g in gorder:
                            cls = _cls(2 * g, u)
                            sct = [scp.tile([128, 1024], f32, tag="sc", name=f"sct{i}") for i in range(2)]
                            for ti in range(2):
                                t = 2 * g + ti
                                for ah in range(2):
                                    nc.tensor.matmul(
                                        sct[ah][:, ti * 512:(ti + 1) * 512],
                                        lhsT=kt_sb[ah * 64:(ah + 1) * 64, hp,
                                                   t * 128:(t + 1) * 128],
                                        rhs=qt_sb[ah * 64:(ah + 1) * 64, hp,
                                                  u * 512:(u + 1) * 512],
                                        start=True, stop=True,
                                        tile_position=(ah * 64, 0),
                                    )
                            for ah in range(2):
                                lh = 2 * hp + ah
                                pt = ppool.tile([128, 1024], f16, tag="pt", bufs=6)
                                nc.scalar.activation(
                                    pt[:], sct[ah][:], AF.Exp,
                                    bias=cb_sb[:, lh, cls:cls + 1], scale=1.0,
                                )
                                if cls == 0 and _lvl >= 2:
                                    src = ppool.tile([128, 1024], f16, tag="pt2", bufs=6)
                                    for ti in range(2):
                                        nc.vector.tensor_mul(
                                            src[:, ti * 512:(ti + 1) * 512],
                                            pt[:, ti * 512:(ti + 1) * 512],
                                            eb_sb[:, lh, _didx(2 * g + ti, u), :],
                                        )
                                else:
                                    src = pt
                                if _lvl >= 3:
                                    for ti in range(2):
                                        t = 2 * g + ti
                                        vsl = v_sb[:, t, :].rearrange(
                                            "p (h x) -> p h x", x=65)[:, ah + 2 * hp, :]
                                        nav[ah] += 1
                                        nc.tensor.matmul(
                                            ctxp[ah][:],
                                            lhsT=vsl,
                                            rhs=src[:, ti * 512:(ti + 1) * 512],
                                            start=(nav[ah] == 1), stop=(nav[ah] == NKT),
                                        )
                                elif g == 0 and ah == 0:
                                    dbg = opool.tile([128, 512], f32, tag="ob", name="dbg")
                                    nc.vector.tensor_copy(dbg[:], src[:, 0:512])
                                    nc.sync.dma_start(
                                        outd[(u * 2 + hp) * 128:
                                             (u * 2 + hp + 1) * 128, :], dbg[:])
                        if _lvl < 3:
                            continue
                        # normalization for both heads of this pair
                        for ah in range(2):
                            ctxf = cpool.tile([65, 512], f32, tag="ctxf", bufs=3)
                            nc.vector.tensor_copy(ctxf[:], ctxp[ah][:])
                            if _lvl < 4:
                                nc.sync.dma_start(
                                    outd[(u * 2 + hp) * 128 + ah * 64:
                                         (u * 2 + hp) * 128 + ah * 64 + 65, :],
                                    ctxf[:],
                                )
                                continue
                            lp0 = cpool.tile([1, 512], f32, tag="lp0")
                            nc.sync.dma_start(lp0[:], ctxf[64:65, :])
                            linv = cpool.tile([1, 512], f32, tag="linv")
                            nc.vector.reciprocal_approx_fast(linv[:], lp0[:])
                            linvb = cpool.tile([1, 512], mybir.dt.float16, tag="linvb")
                            nc.vector.tensor_scalar_mul(linvb[:], linv[:], 256.0)
                            bc = c1p.tile([64, 512], f32, tag="cp", bufs=2)
                            nc.tensor.matmul(bc[:], lhsT=ones_c[:], rhs=linvb[:],
                                             start=True, stop=True)
                            cxn = cpool.tile([64, 512], f32r, tag="cx", bufs=6,
                                             name=f"cx{hp}{ah}")
                            nc.vector.tensor_mul(cxn[:], bc[:], ctxf[0:64, :])
                            cx[2 * hp + ah] = cxn
                        if _lvl == 4:
                            nc.sync.dma_start(
                                outd[(u * 2 + hp) * 128:(u * 2 + hp) * 128 + 64, :],
                                cx[2 * hp][:].bitcast(f32),
                            )
                    if _lvl < 5:
                        continue
                    # O-projection for this q-unit: accumulate all 4 heads
                    for qs in range(4):
                        po = c1p.tile([128, 512], f32, tag="cp", bufs=2)
                        for lh in range(4):
                            nc.tensor.matmul(
                                po[:],
                                lhsT=cx[lh][:, qs * 128:(qs + 1) * 128],
                                rhs=wo_sb[:, lh, :],
                                start=(lh == 0), stop=(lh == 3),
                            )
                        ob = opool.tile([128, 512], f32, tag="ob")
                        nc.vector.tensor_copy(ob[:], po[:])
                        nc.sync.dma_start(
                            outd[u * 512 + qs * 128: u * 512 + (qs + 1) * 128, :],
                            ob[:],
                        )
    nc.compile()
    return nc


_PROGRAM = None


def _get_program():
    global _PROGRAM
    if _PROGRAM is None:
        _PROGRAM = build_program()
    return _PROGRAM


# index table for the in-band Toeplitz bias blocks, shared across heads
_IDX = None


def _idx_table():
    global _IDX
    if _IDX is None:
        p = np.arange(128)[:, None]
        f = np.arange(512)[None, :]
        blocks = []
        for didx in range(8):
            delta = didx * 128 - 256
            blocks.append(np.clip(delta + p - f + 255, 0, 510))
        _IDX = np.stack(blocks, axis=0)  # [8, 128, 512]
    return _IDX


def kernel(**inputs):
    import ml_dtypes

    query = np.asarray(inputs["query"], dtype=np.float32)
    key = np.asarray(inputs["key"], dtype=np.float32)
    value = np.asarray(inputs["value"], dtype=np.float32)
    mask = np.asarray(inputs["mask"])
    Wq = np.asarray(inputs["Wq"], dtype=np.float32)
    Wk = np.asarray(inputs["Wk"], dtype=np.float32)
    Wv = np.asarray(inputs["Wv"], dtype=np.float32)
    Wo = np.asarray(inputs["Wo"], dtype=np.float32)
    bo = np.asarray(inputs["bo"], dtype=np.float32)
    rel_bias = np.asarray(inputs["rel_bias"], dtype=np.float32)

    if not np.all(mask != 0):
        raise NotImplementedError("kernel assumes an all-ones attention mask")

    nc = _get_program()
    idx = _idx_table()
    scale = np.float32(1.0 / np.sqrt(DK))

    in_maps = []
    for c in range(NCORES):
        b = c // 2
        hbase = (c % 2) * 4
        rows = slice(hbase * 64, (hbase + 4) * 64)

        wq_arr = np.ascontiguousarray(
            (Wq[rows, :] * scale).T.reshape(4, 128, 256).swapaxes(0, 1))
        wk_arr = np.ascontiguousarray(
            Wk[rows, :].T.reshape(4, 128, 256).swapaxes(0, 1))
        wv_arr = np.ascontiguousarray(
            Wv[rows, :].T.reshape(4, 128, 256).swapaxes(0, 1))

        wo_arr = np.empty((64, 4, 512), dtype=np.float32)
        eb_arr = np.empty((128, 4, 8, 512), dtype=np.float16)
        cb_arr = np.zeros((128, 4, 3), dtype=np.float32)
        for lh in range(4):
            g = hbase + lh
            wo_arr[:, lh, :] = Wo[:, g * 64:(g + 1) * 64].T * (1.0 / 256.0)
            tbl = rel_bias[g]
            eb_arr[:, lh, :, :] = np.exp(tbl)[idx].transpose(1, 0, 2)
            cb_arr[:, lh, 1] = tbl[0]
            cb_arr[:, lh, 2] = tbl[510]

        bf = np.float16
        in_maps.append({
            "xqT": np.ascontiguousarray(query[b].T).astype(bf),
            "xkT": np.ascontiguousarray(key[b].T).astype(bf),
            "xvT": np.ascontiguousarray(value[b].T).astype(bf),
            "wq": wq_arr.astype(bf), "wk": wk_arr.astype(bf),
            "wv": wv_arr.astype(bf), "wo": wo_arr,
            "eb": eb_arr, "cb": cb_arr,
        })

    res = run_bass_kernel_spmd(nc, in_maps, list(range(NCORES)), trace=False)

    out = np.zeros((B, S, D), dtype=np.float32)
    for c in range(NCORES):
        out[c // 2] += res.results[c]["out"]
    out += bo[None, None, :]
    return out



# revision 5
# speedup vs baseline: 1.2382x; 1.2382x over previous
"""Trainium2 Bass kernel for nn_MultiHeadAttention_34162169872901.

MultiHeadAttention (B=4, S=2048, d_model=512, 8 heads, d_k=64) with a
relative-position bias table (511 entries, clamp +-255) and an all-ones mask.

Sharding (8 NeuronCores): core c handles batch b = c//2 and 4 of the 8 heads
(c%2 selects the head half) -- data parallel on B, tensor parallel on heads.
Each core computes its 4 heads' Q/K/V projections, the full attention for its
batch, normalization, and its partial output projection; the host sums the two
partial outputs per batch (and adds the output bias bo).

v2 -- scheduled for the ACT (exp) bottleneck:
  - The softmax exp on the scalar/ACT engine is the binding resource
    (128 x [128,1024] exps ~= 1.13us each).  The schedule starts exp as
    early as possible and tries to never let ACT starve.
  - Input DMAs are split into halves and priority-ordered so the first
    scores matmul can issue ~9us in; V tiles and the remaining projections
    are emitted as interleaved "filler" between score groups (the PE queue
    is strictly in-order, so placement matters).
  - The relative-bias exp-table is a [128, 4, 1408] sliding-window table
    (every in-band Toeplitz block is a contiguous 512-col slice), 1.4MB
    instead of v1's 4MB block table.
  - PSUM: scores ring 2x[128,1024] (4 banks), AV-accumulator ring
    2x[65,512] (2 banks), epilogue/projection ring 2x[128,512] (2 banks).
  - Per-(u,hp) normalization and the O-projection are pipelined into the
    following attention block as filler items.
  - O-projection runs in f16 (f32r matmuls measured ~2x slower).
"""

import sys
import types

import numpy as np

B = 4
S = 2048
D = 512
NHEAD = 8
DK = 64
NCORES = 8
MAX_REL = 255
NKT = S // 128   # 16 k-tiles
NU = S // 512    # 4 q-units
NG = NKT // 2    # 8 score groups per (u, hp)


def _install_axon_hooks():
    """Provide antenv.axon_hooks (missing in this image) so bass_utils'
    trace path can be used; harmless when tracing is off."""
    try:
        import antenv
    except ImportError:
        return
    try:
        from antenv.axon_hooks import get_axon_ntff_profile_hook  # noqa: F401
        return
    except ImportError:
        pass
    hook = None
    try:
        from trn_agent_boot.trn_boot import _ntff_profile_via_ctypes
        hook = _ntff_profile_via_ctypes("/opt/axon/libaxon_pjrt.so")
    except Exception:
        hook = None
    m = types.ModuleType("antenv.axon_hooks")
    m.get_axon_ntff_profile_hook = lambda: hook
    m.set_axon_ntff_profile_hook = lambda h: None
    sys.modules["antenv.axon_hooks"] = m
    antenv.axon_hooks = m


_install_axon_hooks()

import concourse.bass as bass  # noqa: E402
import concourse.bacc as bacc  # noqa: E402
import concourse.mybir as mybir  # noqa: E402
from concourse import tile  # noqa: E402
from concourse.bass_utils import run_bass_kernel_spmd  # noqa: E402
from concourse.vector_clock import ScopedClock as _ScopedClock  # noqa: E402

f32 = mybir.dt.float32
bf16 = mybir.dt.bfloat16
f16 = mybir.dt.float16
AF = mybir.ActivationFunctionType


def _patched_drain_and_barrier(self, tick_clock, wait_clock):
    # walrus in this container rejects >2 sem waits on one instruction; emit
    # the tail-drain waits as standalone wait instructions instead.
    nc = self.nc
    dummy = mybir.InstNoOp(name="drain-wait-probe", engine=mybir.EngineType.SP)
    wait_clock.add_sem_waits(dummy, _ScopedClock({None: tick_clock.global_clock}))
    handles = {h.name: h for h in self.sems.allocated().values()}
    si = dummy.sync_info
    for w in (si.on_wait if si is not None else []):
        nc.sync.wait_ge(handles[w.ant_name], w.wait_value)
    nc.sync.drain()
    nc.all_engine_barrier()
    popped = nc._tile_sem_poison_stack.pop()
    assert popped is self._sem_poison
    nc.clear_and_free_semaphores(list(self.sems.allocated().values()))
    nc.all_engine_barrier()


tile.TileContext._drain_and_barrier = _patched_drain_and_barrier


def _delta(t, u):
    # key-tile offset minus query-chunk offset; bias entry index is
    # delta + (p - f) + 255 clipped to [0, 510]
    return 128 * t - 512 * u


def _cls(t, u):
    d = _delta(t, u)
    if d <= -384:
        return 1  # whole block clamps to table[0]
    if d >= 768:
        return 2  # whole block clamps to table[510]
    return 0      # in-band: needs the Toeplitz block


def _ebt_col(t, u):
    # start column of the [128,512] in-band block inside the 1408-wide
    # sliding-window exp-bias table: 640 - delta
    return 640 - _delta(t, u)


def _gorder(u):
    # out-of-band groups first (no bias-table dependency, constant folded
    # into the exp bias); for u == 0 prefer high-g groups since the
    # K-projection for columns 1024.. completes first (xk half-1 DMA'd
    # first).
    def key(g):
        inband = _cls(2 * g, u) == 0
        late = 0 if g >= 4 else 1
        return (inband, late if u == 0 else 0, g)
    return sorted(range(NG), key=key)


def build_program():
    nc = bacc.Bacc()

    xqT = nc.declare_dram_parameter("xqT", [D, S], f16, isOutput=False)
    xkT = nc.declare_dram_parameter("xkT", [D, S], f16, isOutput=False)
    xvT = nc.declare_dram_parameter("xvT", [D, S], f16, isOutput=False)
    wq = nc.declare_dram_parameter("wq", [128, 4, 256], f16, isOutput=False)
    wk = nc.declare_dram_parameter("wk", [128, 4, 256], f16, isOutput=False)
    wv = nc.declare_dram_parameter("wv", [128, 4, 256], f16, isOutput=False)
    wo = nc.declare_dram_parameter("wo", [64, 4, 512], f16, isOutput=False)
    ebtd = nc.declare_dram_parameter("ebt", [128, 4, 1408], f16, isOutput=False)
    cbd = nc.declare_dram_parameter("cb", [128, 4, 3], f32, isOutput=False)
    outd = nc.declare_dram_parameter("out", [S, D], f32, isOutput=True)

    with tile.TileContext(nc) as tc:
        with (
            tc.tile_pool(name="sb", bufs=1) as pool,
            tc.tile_pool(name="xt", bufs=1) as xpool,
            tc.tile_pool(name="pt", bufs=12) as ppool,
            tc.tile_pool(name="cxp", bufs=2) as cpool,
            tc.tile_pool(name="obp", bufs=4) as opool,
            tc.tile_pool(name="sc", bufs=2, space="PSUM") as scp,
            tc.tile_pool(name="cx", bufs=2, space="PSUM") as ctxpool,
            tc.tile_pool(name="ep", bufs=2, space="PSUM") as epp,
        ):
            # ---- persistent SBUF tiles -------------------------------------
            wq_sb = pool.tile([128, 4, 256], f16, tag="wq")
            wk_sb = pool.tile([128, 4, 256], f16, tag="wk")
            wv_sb = pool.tile([128, 4, 256], f16, tag="wv")
            wo_sb = pool.tile([64, 4, 512], f16, tag="wo")
            ebt_sb = pool.tile([128, 4, 1408], f16, tag="ebt")
            cb_sb = pool.tile([128, 4, 3], f32, tag="cb")
            qt_sb = pool.tile([128, 2, S], f16, tag="qt")
            kt_sb = pool.tile([128, 2, S], f16, tag="kt")
            v_sb = pool.tile([128, NKT, 4 * 65], f16, tag="v")
            ones_c = pool.tile([1, 64], f16, tag="ones")
            warm = pool.tile([128, 16], f32, tag="warm")

            # load the exp table set immediately (one-time ~2.7us)
            nc.vector.memset(warm[:], 0.0)
            nc.scalar.activation(warm[:], warm[:], AF.Exp, bias=0.0, scale=1.0)
            nc.vector.memset(ones_c[:], 1.0)

            # ---- input DMA triggers, priority ordered ----------------------
            xk_t = {}
            xq_t = {}
            xv_t = [xpool.tile([128, S], f16, tag=f"xv{ct}", name=f"xv{ct}")
                    for ct in range(4)]
            for ct in range(4):
                for h in range(2):
                    xk_t[(ct, h)] = xpool.tile(
                        [128, 1024], f16, tag=f"xk{ct}{h}", name=f"xk{ct}{h}")
                    xq_t[(ct, h)] = xpool.tile(
                        [128, 1024], f16, tag=f"xq{ct}{h}", name=f"xq{ct}{h}")
            # sync queue: K/Q weights, xk half-1 (k cols 1024:), two V tiles,
            # xk half-0
            nc.sync.dma_start(wq_sb[:], wq[:])
            nc.sync.dma_start(wk_sb[:], wk[:])
            for ct in range(4):
                nc.sync.dma_start(
                    xk_t[(ct, 1)][:], xkT[ct * 128:(ct + 1) * 128, 1024:2048])
            nc.sync.dma_start(xv_t[0][:], xvT[0:128, :])
            nc.sync.dma_start(xv_t[1][:], xvT[128:256, :])
            for ct in range(4):
                nc.sync.dma_start(
                    xk_t[(ct, 0)][:], xkT[ct * 128:(ct + 1) * 128, 0:1024])
            # scalar queue (all triggers fire before the first exp): V/O
            # weights + bias consts (small), xq half-0
            nc.scalar.dma_start(wv_sb[:], wv[:])
            nc.scalar.dma_start(wo_sb[:], wo[:])
            nc.scalar.dma_start(cb_sb[:], cbd[:])
            for ct in range(4):
                nc.scalar.dma_start(
                    xq_t[(ct, 0)][:], xqT[ct * 128:(ct + 1) * 128, 0:1024])
            # gpsimd queue: remaining V tiles, the exp-bias window table,
            # xq half-1
            nc.gpsimd.dma_start(xv_t[2][:], xvT[256:384, :])
            nc.gpsimd.dma_start(xv_t[3][:], xvT[384:512, :])
            nc.gpsimd.dma_start(ebt_sb[:, 0:2, :], ebtd[:, 0:2, :])
            nc.gpsimd.dma_start(ebt_sb[:, 2:4, :], ebtd[:, 2:4, :])
            for ct in range(4):
                nc.gpsimd.dma_start(
                    xq_t[(ct, 1)][:], xqT[ct * 128:(ct + 1) * 128, 1024:2048])

            # ---- projection helpers ----------------------------------------
            def proj_group(w_sb, x_tiles, dst, hp, sc, ptag):
                pk = (ctxpool if ptag == "cx" else epp).tile(
                    [128, 512], f32, tag=ptag, name=f"pj{hp}{sc}")
                h = sc // 2
                col = (sc % 2) * 512
                for ct in range(4):
                    nc.tensor.matmul(
                        pk[:],
                        lhsT=w_sb[:, ct, hp * 128:(hp + 1) * 128],
                        rhs=x_tiles[(ct, h)][:, col:col + 512],
                        start=(ct == 0), stop=(ct == 3),
                    )
                nc.vector.tensor_copy(dst[:, hp, sc * 512:(sc + 1) * 512], pk[:])

            # head-pair 0, first halves: Q cols 0:1024 and K cols 1024:2048
            for sc in (0, 1):
                proj_group(wq_sb, xq_t, qt_sb, 0, sc, "ep")
            for sc in (2, 3):
                proj_group(wk_sb, xk_t, kt_sb, 0, sc, "cx")

            def v_group(g):
                def emit():
                    pv = epp.tile([128, 512], f32, tag="ep", name=f"pv{g}")
                    for sti in range(2):
                        st = 2 * g + sti
                        for ct in range(4):
                            nc.tensor.matmul(
                                pv[:, sti * 256:sti * 256 + 256],
                                lhsT=xv_t[ct][:, st * 128:(st + 1) * 128],
                                rhs=wv_sb[:, ct, :],
                                start=(ct == 0), stop=(ct == 3),
                            )
                    for sti in range(2):
                        st = 2 * g + sti
                        vslice = v_sb[:, st, :].rearrange(
                            "p (h x) -> p h x", x=65)
                        nc.vector.tensor_copy(
                            vslice[:, :, 0:64],
                            pv[:, sti * 256:sti * 256 + 256].rearrange(
                                "p (h x) -> p h x", x=64),
                        )
                        nc.vector.memset(vslice[:, :, 64:65], 1.0)
                return emit

            def pg_item(w_sb, x_tiles, dst, hp, sc):
                def emit():
                    proj_group(w_sb, x_tiles, dst, hp, sc, "ep")
                return emit

            # ---- attention + pipelined epilogue ----------------------------
            cx_tiles = {}     # (u, lh) -> normalized ctx [64, 512] f16
            filler = []       # FIFO of emission closures

            def epilogue_items(u, hp, ctxps):
                """Normalization for the two heads of (u, hp), split into
                pipeline-friendly chunks."""
                state = {}

                def e1():
                    for ah in range(2):
                        ctxf = cpool.tile([65, 512], f32, tag="ctxf", bufs=4,
                                          name=f"ctxf{u}{hp}{ah}")
                        nc.vector.tensor_copy(ctxf[:], ctxps[ah][:])
                        state[ah] = ctxf

                def e2a():
                    lp = cpool.tile([1, 1024], f32, tag="lp", name=f"lp{u}{hp}")
                    nc.gpsimd.dma_start(lp[:, 0:512], state[0][64:65, :])
                    nc.gpsimd.dma_start(lp[:, 512:1024], state[1][64:65, :])
                    state["lp"] = lp

                def e2b():
                    linv = cpool.tile([1, 1024], f32, tag="linv",
                                      name=f"linv{u}{hp}")
                    nc.vector.reciprocal_approx_fast(linv[:], state["lp"][:])
                    linvb = cpool.tile([1, 1024], f16, tag="linvb",
                                       name=f"linvb{u}{hp}")
                    nc.vector.tensor_scalar_mul(linvb[:], linv[:], 256.0)
                    state["linvb"] = linvb

                def e3():
                    linvb = state["linvb"]
                    for ah in range(2):
                        bc = epp.tile([64, 512], f32, tag="ep",
                                      name=f"bc{u}{hp}{ah}")
                        nc.tensor.matmul(
                            bc[:], lhsT=ones_c[:],
                            rhs=linvb[:, ah * 512:(ah + 1) * 512],
                            start=True, stop=True)
                        cxn = cpool.tile([64, 512], f16, tag="cxn", bufs=8,
                                         name=f"cx{u}{hp}{ah}")
                        nc.vector.tensor_mul(cxn[:], bc[:], state[ah][0:64, :])
                        cx_tiles[(u, 2 * hp + ah)] = cxn

                return [e1, e2a, e2b, e3]

            def oproj_items(u):
                items = []
                for qs in range(4):
                    def emit(u=u, qs=qs):
                        po = epp.tile([128, 512], f32, tag="ep",
                                      name=f"po{u}{qs}")
                        for lh in range(4):
                            nc.tensor.matmul(
                                po[:],
                                lhsT=cx_tiles[(u, lh)][:, qs * 128:(qs + 1) * 128],
                                rhs=wo_sb[:, lh, :],
                                start=(lh == 0), stop=(lh == 3),
                            )
                        ob = opool.tile([128, 512], f32, tag="ob",
                                        name=f"ob{u}{qs}")
                        nc.vector.tensor_copy(ob[:], po[:])
                        nc.sync.dma_start(
                            outd[u * 512 + qs * 128: u * 512 + (qs + 1) * 128, :],
                            ob[:],
                        )
                    items.append(emit)
                return items

            def attention(u, hp, pre_sched=None, post_sched=None, post_rate=1):
                """pre_sched/post_sched: {gi: [closures]} emitted before the
                scores (pre) or between the exps and the AV matmuls (post) of
                group gi.  post_rate: how many queued filler items to pop at
                each post point (in addition to post_sched)."""
                ctxps = [
                    ctxpool.tile([65, 512], f32, tag="cx",
                                 name=f"ctxp{u}{hp}{i}")
                    for i in range(2)
                ]
                nav = [0, 0]
                for gi, g in enumerate(_gorder(u)):
                    for fn in (pre_sched or {}).get(gi, []):
                        fn()
                    cls = _cls(2 * g, u)
                    sct = [scp.tile([128, 1024], f32, tag="sc",
                                    name=f"sct{u}{hp}{g}{i}")
                           for i in range(2)]
                    for ti in range(2):
                        t = 2 * g + ti
                        for ah in range(2):
                            nc.tensor.matmul(
                                sct[ah][:, ti * 512:(ti + 1) * 512],
                                lhsT=kt_sb[ah * 64:(ah + 1) * 64, hp,
                                           t * 128:(t + 1) * 128],
                                rhs=qt_sb[ah * 64:(ah + 1) * 64, hp,
                                          u * 512:(u + 1) * 512],
                                start=True, stop=True,
                                tile_position=(ah * 64, 0),
                            )
                    pts = []
                    for ah in range(2):
                        lh = 2 * hp + ah
                        pt = ppool.tile([128, 1024], f16, tag="pt",
                                        name=f"pt{u}{hp}{g}{ah}")
                        nc.scalar.activation(
                            pt[:], sct[ah][:], AF.Exp,
                            bias=cb_sb[:, lh, cls:cls + 1], scale=1.0,
                        )
                        pts.append(pt)
                    for fn in (post_sched or {}).get(gi, []):
                        fn()
                    for _ in range(post_rate):
                        if filler:
                            filler.pop(0)()
                    for ah in range(2):
                        lh = 2 * hp + ah
                        pt = pts[ah]
                        if cls == 0:
                            src = ppool.tile([128, 1024], f16, tag="src",
                                             bufs=8,
                                             name=f"src{u}{hp}{g}{ah}")
                            for ti in range(2):
                                col = _ebt_col(2 * g + ti, u)
                                nc.vector.tensor_mul(
                                    src[:, ti * 512:(ti + 1) * 512],
                                    pt[:, ti * 512:(ti + 1) * 512],
                                    ebt_sb[:, lh, col:col + 512],
                                )
                        else:
                            src = pt
                        for ti in range(2):
                            t = 2 * g + ti
                            vsl = v_sb[:, t, :].rearrange(
                                "p (h x) -> p h x", x=65)[:, lh, :]
                            nav[ah] += 1
                            nc.tensor.matmul(
                                ctxps[ah][:],
                                lhsT=vsl,
                                rhs=src[:, ti * 512:(ti + 1) * 512],
                                start=(nav[ah] == 1), stop=(nav[ah] == NKT),
                            )
                return ctxps

            # block (0,0): V-projections + remaining K/Q projections are
            # placed explicitly.  gorder(0) = [4,5,6,7,3,0,1,2]:
            #  - scores(gi4)=g3 needs kt cols 768:1024 -> K-hp0 sc1/sc0
            #    emitted pre-scores at gi4 (xk half-0 arrives ~20us).
            #  - V group for gorder[k] is emitted at the post point of
            #    gi<=k (AV of that group follows it in program order).
            pre00 = {4: [pg_item(wk_sb, xk_t, kt_sb, 0, 1),
                         pg_item(wk_sb, xk_t, kt_sb, 0, 0)]}
            post00 = {
                0: [v_group(4)],
                1: [v_group(5)],
                2: [v_group(6)],
                3: [v_group(7)],
                4: [v_group(3)],
                5: [v_group(0),
                    pg_item(wk_sb, xk_t, kt_sb, 1, 0),
                    pg_item(wk_sb, xk_t, kt_sb, 1, 1)],
                6: [v_group(1),
                    pg_item(wk_sb, xk_t, kt_sb, 1, 2),
                    pg_item(wk_sb, xk_t, kt_sb, 1, 3)],
                7: [v_group(2),
                    pg_item(wq_sb, xq_t, qt_sb, 1, 0),
                    pg_item(wq_sb, xq_t, qt_sb, 1, 1),
                    pg_item(wq_sb, xq_t, qt_sb, 1, 2),
                    pg_item(wq_sb, xq_t, qt_sb, 1, 3)],
            }
            ctxps = attention(0, 0, pre00, post00, post_rate=0)
            filler.extend([pg_item(wq_sb, xq_t, qt_sb, 0, 2),
                           pg_item(wq_sb, xq_t, qt_sb, 0, 3)])
            filler.extend(epilogue_items(0, 0, ctxps))

            for u in range(NU):
                for hp in range(2):
                    if u == 0 and hp == 0:
                        continue
                    ctxps = attention(u, hp, post_rate=1)
                    filler.extend(epilogue_items(u, hp, ctxps))
                    if hp == 1:
                        filler.extend(oproj_items(u))
            while filler:
                filler.pop(0)()

    nc.compile()
    return nc


_PROGRAM = None


def _get_program():
    global _PROGRAM
    if _PROGRAM is None:
        _PROGRAM = build_program()
    return _PROGRAM


def kernel(**inputs):
    query = np.asarray(inputs["query"], dtype=np.float32)
    key = np.asarray(inputs["key"], dtype=np.float32)
    value = np.asarray(inputs["value"], dtype=np.float32)
    mask = np.asarray(inputs["mask"])
    Wq = np.asarray(inputs["Wq"], dtype=np.float32)
    Wk = np.asarray(inputs["Wk"], dtype=np.float32)
    Wv = np.asarray(inputs["Wv"], dtype=np.float32)
    Wo = np.asarray(inputs["Wo"], dtype=np.float32)
    bo = np.asarray(inputs["bo"], dtype=np.float32)
    rel_bias = np.asarray(inputs["rel_bias"], dtype=np.float32)

    if not np.all(mask != 0):
        raise NotImplementedError("kernel assumes an all-ones attention mask")

    nc = _get_program()
    scale = np.float32(1.0 / np.sqrt(DK))

    # sliding-window exp-bias table: ebt[p, lh, j] = exp(tbl[clip(895-j+p)])
    pp = np.arange(128)[:, None]
    jj = np.arange(1408)[None, :]
    widx = np.clip(895 - jj + pp, 0, 510)  # [128, 1408]

    in_maps = []
    for c in range(NCORES):
        b = c // 2
        hbase = (c % 2) * 4
        rows = slice(hbase * 64, (hbase + 4) * 64)

        wq_arr = np.ascontiguousarray(
            (Wq[rows, :] * scale).T.reshape(4, 128, 256).swapaxes(0, 1))
        wk_arr = np.ascontiguousarray(
            Wk[rows, :].T.reshape(4, 128, 256).swapaxes(0, 1))
        wv_arr = np.ascontiguousarray(
            Wv[rows, :].T.reshape(4, 128, 256).swapaxes(0, 1))

        wo_arr = np.empty((64, 4, 512), dtype=np.float32)
        ebt_arr = np.empty((128, 4, 1408), dtype=np.float16)
        cb_arr = np.zeros((128, 4, 3), dtype=np.float32)
        for lh in range(4):
            g = hbase + lh
            wo_arr[:, lh, :] = Wo[:, g * 64:(g + 1) * 64].T * (1.0 / 256.0)
            tbl = rel_bias[g]
            ebt_arr[:, lh, :] = np.exp(tbl)[widx]
            cb_arr[:, lh, 1] = tbl[0]
            cb_arr[:, lh, 2] = tbl[510]

        bf = np.float16
        in_maps.append({
            "xqT": np.ascontiguousarray(query[b].T).astype(bf),
            "xkT": np.ascontiguousarray(key[b].T).astype(bf),
            "xvT": np.ascontiguousarray(value[b].T).astype(bf),
            "wq": wq_arr.astype(bf), "wk": wk_arr.astype(bf),
            "wv": wv_arr.astype(bf), "wo": wo_arr.astype(bf),
            "ebt": ebt_arr, "cb": cb_arr,
        })

    res = run_bass_kernel_spmd(nc, in_maps, list(range(NCORES)), trace=False)

    out = np.zeros((B, S, D), dtype=np.float32)
    for c in range(NCORES):
        out[c // 2] += res.results[c]["out"]
    out += bo[None, None, :]
    return out


# revision 8
# speedup vs baseline: 1.3078x; 1.0562x over previous
"""Trainium2 Bass kernel for nn_MultiHeadAttention_34162169872901.

MultiHeadAttention (B=4, S=2048, d_model=512, 8 heads, d_k=64) with a
relative-position bias table (511 entries, clamp +-255) and an all-ones mask.

Sharding (8 NeuronCores): core c handles batch b = c//2 and 4 of the 8 heads
(c%2 selects the head half) -- data parallel on B, tensor parallel on heads.
Each core computes its 4 heads' Q/K/V projections, the full attention for its
batch, normalization, and its partial output projection; the host sums the two
partial outputs per batch (and adds the output bias bo).

v3 -- scheduled for the ACT (exp) bottleneck:
  - The softmax exp on the scalar/ACT engine is the binding resource
    (128 x [128,1024] exps ~= 1.13us each).  The schedule starts exp as
    early as possible (~7us) and tries to never let ACT starve.
  - Inputs are tiled by first-use (xq per 512-col q-unit; xk in 1024/512/512
    col pieces) and DMA'd in need order on the two fast trigger queues
    (sync + gpsimd; the scalar-triggered queue is slow and is the
    bottleneck engine anyway).
  - The relative-bias exp-table is a [128, 4, 1408] sliding-window table
    (every in-band Toeplitz block is a contiguous 512-col slice).
  - PSUM: scores ring 2x[128,1024] (4 banks), AV-accumulator ring
    2x[65,512] (2 banks), epilogue/projection ring 2x[128,512] (2 banks).
  - V projection, remaining K/Q projections, per-u normalization and the
    O-projection are emitted as filler between score groups (the PE queue
    is strictly in-order, so placement matters); AV matmuls are emitted
    with a small lag so filler never lands between a score group and the
    exp that frees its PSUM bank.
  - The final O-projection is split: the first two heads' partials are
    computed and staged during the last attention block, and junk
    keep-warm matmuls bridge the final normalization chain so the tail
    matmuls run at full clock.
"""

import sys
import types

import numpy as np

B = 4
S = 2048
D = 512
NHEAD = 8
DK = 64
NCORES = 8
MAX_REL = 255
NKT = S // 128   # 16 k-tiles
NU = S // 512    # 4 q-units
NG = NKT // 2    # 8 score groups per (u, hp)


def _install_axon_hooks():
    """Provide antenv.axon_hooks (missing in this image) so bass_utils'
    trace path can be used; harmless when tracing is off."""
    try:
        import antenv
    except ImportError:
        return
    try:
        from antenv.axon_hooks import get_axon_ntff_profile_hook  # noqa: F401
        return
    except ImportError:
        pass
    hook = None
    try:
        from trn_agent_boot.trn_boot import _ntff_profile_via_ctypes
        hook = _ntff_profile_via_ctypes("/opt/axon/libaxon_pjrt.so")
    except Exception:
        hook = None
    m = types.ModuleType("antenv.axon_hooks")
    m.get_axon_ntff_profile_hook = lambda: hook
    m.set_axon_ntff_profile_hook = lambda h: None
    sys.modules["antenv.axon_hooks"] = m
    antenv.axon_hooks = m


_install_axon_hooks()

import concourse.bass as bass  # noqa: E402
import concourse.bacc as bacc  # noqa: E402
import concourse.mybir as mybir  # noqa: E402
from concourse import tile  # noqa: E402
from concourse.bass_utils import run_bass_kernel_spmd  # noqa: E402
from concourse.vector_clock import ScopedClock as _ScopedClock  # noqa: E402

f32 = mybir.dt.float32
bf16 = mybir.dt.bfloat16
f16 = mybir.dt.float16
AF = mybir.ActivationFunctionType


def _patched_drain_and_barrier(self, tick_clock, wait_clock):
    # walrus in this container rejects >2 sem waits on one instruction; emit
    # the tail-drain waits as standalone wait instructions instead.
    nc = self.nc
    dummy = mybir.InstNoOp(name="drain-wait-probe", engine=mybir.EngineType.SP)
    wait_clock.add_sem_waits(dummy, _ScopedClock({None: tick_clock.global_clock}))
    handles = {h.name: h for h in self.sems.allocated().values()}
    si = dummy.sync_info
    for w in (si.on_wait if si is not None else []):
        nc.sync.wait_ge(handles[w.ant_name], w.wait_value)
    nc.sync.drain()
    nc.all_engine_barrier()
    popped = nc._tile_sem_poison_stack.pop()
    assert popped is self._sem_poison
    nc.clear_and_free_semaphores(list(self.sems.allocated().values()))
    nc.all_engine_barrier()


tile.TileContext._drain_and_barrier = _patched_drain_and_barrier


def _delta(t, u):
    # key-tile offset minus query-chunk offset; bias entry index is
    # delta + (p - f) + 255 clipped to [0, 510]
    return 128 * t - 512 * u


def _cls(t, u):
    d = _delta(t, u)
    if d <= -384:
        return 1  # whole block clamps to table[0]
    if d >= 768:
        return 2  # whole block clamps to table[510]
    return 0      # in-band: needs the Toeplitz block


def _ebt_col(t, u):
    # start column of the [128,512] in-band block inside the 1408-wide
    # sliding-window exp-bias table: 640 - delta
    return 640 - _delta(t, u)


def _gorder(u):
    # out-of-band groups first (no bias-table dependency, constant folded
    # into the exp bias).  For u == 0, order by K-projection availability:
    # high k-columns first, in-band groups in descending g.
    if u == 0:
        return [4, 5, 6, 7, 3, 2, 1, 0]
    def key(g):
        return (_cls(2 * g, u) == 0, g)
    return sorted(range(NG), key=key)


def build_program():
    nc = bacc.Bacc()

    xqT = nc.declare_dram_parameter("xqT", [D, S], f16, isOutput=False)
    xkT = nc.declare_dram_parameter("xkT", [D, S], f16, isOutput=False)
    xvT = nc.declare_dram_parameter("xvT", [D, S], f16, isOutput=False)
    wq = nc.declare_dram_parameter("wq", [128, 4, 256], f16, isOutput=False)
    wk = nc.declare_dram_parameter("wk", [128, 4, 256], f16, isOutput=False)
    wv = nc.declare_dram_parameter("wv", [128, 4, 256], f16, isOutput=False)
    wo = nc.declare_dram_parameter("wo", [64, 4, 512], f16, isOutput=False)
    ebtd = nc.declare_dram_parameter("ebt", [128, 4, 1408], f16, isOutput=False)
    cbd = nc.declare_dram_parameter("cb", [128, 4, 3], f32, isOutput=False)
    outd = nc.declare_dram_parameter("out", [S, D], f32, isOutput=True)

    with tile.TileContext(nc) as tc:
        with (
            tc.tile_pool(name="sb", bufs=1) as pool,
            tc.tile_pool(name="xt", bufs=1) as xpool,
            tc.tile_pool(name="pt", bufs=14) as ppool,
            tc.tile_pool(name="cxp", bufs=2) as cpool,
            tc.tile_pool(name="obp", bufs=4) as opool,
            tc.tile_pool(name="sc", bufs=2, space="PSUM") as scp,
            tc.tile_pool(name="cx", bufs=2, space="PSUM") as ctxpool,
            tc.tile_pool(name="ep", bufs=2, space="PSUM") as epp,
        ):
            # ---- persistent SBUF tiles -------------------------------------
            wq_sb = pool.tile([128, 4, 256], f16, tag="wq")
            wk_sb = pool.tile([128, 4, 256], f16, tag="wk")
            wv_sb = pool.tile([128, 4, 256], f16, tag="wv")
            wo_sb = pool.tile([64, 4, 512], f16, tag="wo")
            ebt_sb = pool.tile([128, 4, 1408], f16, tag="ebt")
            cb_sb = pool.tile([128, 4, 3], f32, tag="cb")
            qt_sb = pool.tile([128, 2, S], f16, tag="qt")
            kt_sb = pool.tile([128, 2, S], f16, tag="kt")
            v_sb = pool.tile([128, NKT, 4 * 65], f16, tag="v")
            ones_c = pool.tile([1, 64], f16, tag="ones")
            warm = pool.tile([128, 16], f32, tag="warm")

            # load the exp table set immediately (one-time ~2.7us)
            nc.vector.memset(warm[:], 0.0)
            nc.scalar.activation(warm[:], warm[:], AF.Exp, bias=0.0, scale=1.0)
            nc.vector.memset(ones_c[:], 1.0)

            # ---- input tiles ----------------------------------------------
            # xq: per (ct, q-unit) [128,512]; xk: per ct in three pieces
            # (a: cols 1024:2048, b: 512:1024, c: 0:512); xv: per ct full.
            xq_t = {(ct, un): xpool.tile([128, 512], f16, tag=f"xq{ct}{un}",
                                         name=f"xq{ct}{un}")
                    for ct in range(4) for un in range(4)}
            xka = [xpool.tile([128, 1024], f16, tag=f"xka{ct}",
                              name=f"xka{ct}") for ct in range(4)]
            xkb = [xpool.tile([128, 512], f16, tag=f"xkb{ct}",
                              name=f"xkb{ct}") for ct in range(4)]
            xkc = [xpool.tile([128, 512], f16, tag=f"xkc{ct}",
                              name=f"xkc{ct}") for ct in range(4)]
            xv_t = [xpool.tile([128, S], f16, tag=f"xv{ct}", name=f"xv{ct}")
                    for ct in range(4)]

            # sync queue, in need order
            nc.sync.dma_start(wq_sb[:], wq[:])
            nc.sync.dma_start(wv_sb[:], wv[:])
            for ct in range(4):
                nc.sync.dma_start(xq_t[(ct, 0)][:],
                                  xqT[ct * 128:(ct + 1) * 128, 0:512])
            nc.sync.dma_start(xv_t[0][:], xvT[0:128, :])
            nc.sync.dma_start(xv_t[1][:], xvT[128:256, :])
            for ct in range(4):
                nc.sync.dma_start(xkb[ct][:],
                                  xkT[ct * 128:(ct + 1) * 128, 512:1024])
            for ct in range(4):
                nc.sync.dma_start(xkc[ct][:],
                                  xkT[ct * 128:(ct + 1) * 128, 0:512])
            for un in range(1, 4):
                for ct in range(4):
                    nc.sync.dma_start(
                        xq_t[(ct, un)][:],
                        xqT[ct * 128:(ct + 1) * 128, un * 512:(un + 1) * 512])
            nc.sync.dma_start(wo_sb[:], wo[:])
            # gpsimd queue, in need order
            nc.gpsimd.dma_start(wk_sb[:], wk[:])
            nc.gpsimd.dma_start(cb_sb[:], cbd[:])
            for ct in range(4):
                nc.gpsimd.dma_start(xka[ct][:],
                                    xkT[ct * 128:(ct + 1) * 128, 1024:2048])
            nc.gpsimd.dma_start(xv_t[2][:], xvT[256:384, :])
            nc.gpsimd.dma_start(xv_t[3][:], xvT[384:512, :])
            nc.gpsimd.dma_start(ebt_sb[:, 0:2, :], ebtd[:, 0:2, :])
            nc.gpsimd.dma_start(ebt_sb[:, 2:4, :], ebtd[:, 2:4, :])

            # ---- projection helpers ----------------------------------------
            def k_rhs(sc, ct):
                if sc == 0:
                    return xkc[ct][:, 0:512]
                if sc == 1:
                    return xkb[ct][:, 0:512]
                return xka[ct][:, (sc - 2) * 512:(sc - 1) * 512]

            def q_rhs(sc, ct):
                return xq_t[(ct, sc)][:, 0:512]

            def proj_group(w_sb, rhs_fn, dst, hp, sc, ptag):
                pk = (ctxpool if ptag == "cx" else epp).tile(
                    [128, 512], f32, tag=ptag, name=f"pj{hp}{sc}")
                for ct in range(4):
                    nc.tensor.matmul(
                        pk[:],
                        lhsT=w_sb[:, ct, hp * 128:(hp + 1) * 128],
                        rhs=rhs_fn(sc, ct),
                        start=(ct == 0), stop=(ct == 3),
                    )
                nc.vector.tensor_copy(dst[:, hp, sc * 512:(sc + 1) * 512], pk[:])

            # pre-loop projections: Q hp0 unit-0 and K hp0 high columns
            proj_group(wq_sb, q_rhs, qt_sb, 0, 0, "ep")
            proj_group(wk_sb, k_rhs, kt_sb, 0, 2, "cx")
            proj_group(wk_sb, k_rhs, kt_sb, 0, 3, "cx")

            def v_group(g):
                def emit():
                    pv = epp.tile([128, 512], f32, tag="ep", name=f"pv{g}")
                    for sti in range(2):
                        st = 2 * g + sti
                        for ct in range(4):
                            nc.tensor.matmul(
                                pv[:, sti * 256:sti * 256 + 256],
                                lhsT=xv_t[ct][:, st * 128:(st + 1) * 128],
                                rhs=wv_sb[:, ct, :],
                                start=(ct == 0), stop=(ct == 3),
                            )
                    for sti in range(2):
                        st = 2 * g + sti
                        vslice = v_sb[:, st, :].rearrange(
                            "p (h x) -> p h x", x=65)
                        nc.vector.tensor_copy(
                            vslice[:, :, 0:64],
                            pv[:, sti * 256:sti * 256 + 256].rearrange(
                                "p (h x) -> p h x", x=64),
                        )
                        nc.vector.memset(vslice[:, :, 64:65], 1.0)
                return emit

            def pg_item(w_sb, rhs_fn, dst, hp, sc):
                def emit():
                    proj_group(w_sb, rhs_fn, dst, hp, sc, "ep")
                return emit

            def noop():
                pass

            # ---- attention + pipelined epilogue ----------------------------
            cx_tiles = {}     # (u, lh) -> normalized ctx [64, 512] f16
            ob_a = {}         # qs -> staged hp0 O-projection partial (f32)
            filler = []       # FIFO of emission closures

            def epilogue_items(u, hp, ctxps):
                """Normalization for the two heads of (u, hp), split into
                pipeline-friendly chunks (with no-op spacers so the serial
                chain never blocks an engine queue)."""
                state = {}

                def e1():
                    for ah in range(2):
                        ctxf = cpool.tile([65, 512], f32, tag="ctxf", bufs=4,
                                          name=f"ctxf{u}{hp}{ah}")
                        nc.vector.tensor_copy(ctxf[:], ctxps[ah][:])
                        state[ah] = ctxf

                def e2a():
                    lp = cpool.tile([1, 1024], f32, tag="lp", name=f"lp{u}{hp}")
                    nc.gpsimd.dma_start(lp[:, 0:512], state[0][64:65, :])
                    nc.gpsimd.dma_start(lp[:, 512:1024], state[1][64:65, :])
                    state["lp"] = lp

                def e2b():
                    linv = cpool.tile([1, 1024], f32, tag="linv",
                                      name=f"linv{u}{hp}")
                    nc.vector.reciprocal_approx_fast(linv[:], state["lp"][:])
                    linvb = cpool.tile([1, 1024], f16, tag="linvb",
                                       name=f"linvb{u}{hp}")
                    nc.vector.tensor_scalar_mul(linvb[:], linv[:], 256.0)
                    state["linvb"] = linvb

                def e3():
                    linvb = state["linvb"]
                    for ah in range(2):
                        bc = epp.tile([64, 512], f32, tag="ep",
                                      name=f"bc{u}{hp}{ah}")
                        nc.tensor.matmul(
                            bc[:], lhsT=ones_c[:],
                            rhs=linvb[:, ah * 512:(ah + 1) * 512],
                            start=True, stop=True)
                        cxn = cpool.tile([64, 512], f16, tag="cxn", bufs=8,
                                         name=f"cx{u}{hp}{ah}")
                        nc.vector.tensor_mul(cxn[:], bc[:], state[ah][0:64, :])
                        cx_tiles[(u, 2 * hp + ah)] = cxn

                return [e1, e2a, noop, e2b, noop, e3]

            def oproj_items(u):
                items = []
                for qs in range(4):
                    def emit(u=u, qs=qs):
                        po = epp.tile([128, 512], f32, tag="ep",
                                      name=f"po{u}{qs}")
                        for lh in range(4):
                            nc.tensor.matmul(
                                po[:],
                                lhsT=cx_tiles[(u, lh)][:, qs * 128:(qs + 1) * 128],
                                rhs=wo_sb[:, lh, :],
                                start=(lh == 0), stop=(lh == 3),
                            )
                        ob = opool.tile([128, 512], f32, tag="ob",
                                        name=f"ob{u}{qs}")
                        nc.vector.tensor_copy(ob[:], po[:])
                        nc.sync.dma_start(
                            outd[u * 512 + qs * 128: u * 512 + (qs + 1) * 128, :],
                            ob[:],
                        )
                    items.append(emit)
                return items

            # split O-projection for the last u: stage the hp0-heads partial
            # during the last attention block, finish + combine in the tail
            def oproj_a_item(u, qs):
                def emit():
                    po = epp.tile([128, 512], f32, tag="ep",
                                  name=f"poa{u}{qs}")
                    for lh in range(2):
                        nc.tensor.matmul(
                            po[:],
                            lhsT=cx_tiles[(u, lh)][:, qs * 128:(qs + 1) * 128],
                            rhs=wo_sb[:, lh, :],
                            start=(lh == 0), stop=(lh == 1),
                        )
                    oba = opool.tile([128, 512], f32, tag="oba", bufs=4,
                                     name=f"oba{u}{qs}")
                    nc.vector.tensor_copy(oba[:], po[:])
                    ob_a[qs] = oba
                return emit

            def oproj_b_item(u, qs):
                def emit():
                    po = epp.tile([128, 512], f32, tag="ep",
                                  name=f"pob{u}{qs}")
                    for lh in range(2, 4):
                        nc.tensor.matmul(
                            po[:],
                            lhsT=cx_tiles[(u, lh)][:, qs * 128:(qs + 1) * 128],
                            rhs=wo_sb[:, lh, :],
                            start=(lh == 2), stop=(lh == 3),
                        )
                    ob = opool.tile([128, 512], f32, tag="ob",
                                    name=f"ob{u}{qs}")
                    nc.vector.tensor_add(ob[:], po[:], ob_a[qs][:])
                    nc.sync.dma_start(
                        outd[u * 512 + qs * 128: u * 512 + (qs + 1) * 128, :],
                        ob[:],
                    )
                return emit

            def keep_warm():
                # junk matmuls bridging the tail normalization chain so HAM
                # doesn't re-throttle the PE before the final O-projection
                dz = ctxpool.tile([65, 512], f32, tag="cx", name="dz")
                for r in range(12):
                    vsl = v_sb[:, r, :].rearrange(
                        "p (h x) -> p h x", x=65)[:, 0, :]
                    nc.tensor.matmul(dz[:], lhsT=vsl, rhs=qt_sb[:, 0, 0:512],
                                     start=True, stop=True)

            def attention(u, hp, pre_sched=None, post_sched=None, post_rate=1,
                          av_lag=0):
                """pre_sched/post_sched: {gi: [closures]} emitted before the
                scores (pre) or between the exps and the AV matmuls (post) of
                group gi.  post_rate: queued filler items popped at each post
                point.  av_lag: emit each group's AV matmuls that many groups
                later (so filler with unmet input DMAs can sit between a
                score group and its AV without blocking the score stream)."""
                ctxps = [
                    ctxpool.tile([65, 512], f32, tag="cx",
                                 name=f"ctxp{u}{hp}{i}")
                    for i in range(2)
                ]
                nav = [0, 0]
                avq = []

                def av_emit(g, srcs):
                    for ah in range(2):
                        lh = 2 * hp + ah
                        for ti in range(2):
                            t = 2 * g + ti
                            vsl = v_sb[:, t, :].rearrange(
                                "p (h x) -> p h x", x=65)[:, lh, :]
                            nav[ah] += 1
                            nc.tensor.matmul(
                                ctxps[ah][:],
                                lhsT=vsl,
                                rhs=srcs[ah][:, ti * 512:(ti + 1) * 512],
                                start=(nav[ah] == 1), stop=(nav[ah] == NKT),
                            )

                for gi, g in enumerate(_gorder(u)):
                    for fn in (pre_sched or {}).get(gi, []):
                        fn()
                    cls = _cls(2 * g, u)
                    sct = [scp.tile([128, 1024], f32, tag="sc",
                                    name=f"sct{u}{hp}{g}{i}")
                           for i in range(2)]
                    for ti in range(2):
                        t = 2 * g + ti
                        for ah in range(2):
                            nc.tensor.matmul(
                                sct[ah][:, ti * 512:(ti + 1) * 512],
                                lhsT=kt_sb[ah * 64:(ah + 1) * 64, hp,
                                           t * 128:(t + 1) * 128],
                                rhs=qt_sb[ah * 64:(ah + 1) * 64, hp,
                                          u * 512:(u + 1) * 512],
                                start=True, stop=True,
                                tile_position=(ah * 64, 0),
                            )
                    srcs = []
                    for ah in range(2):
                        lh = 2 * hp + ah
                        pt = ppool.tile([128, 1024], f16, tag="pt",
                                        name=f"pt{u}{hp}{g}{ah}")
                        nc.scalar.activation(
                            pt[:], sct[ah][:], AF.Exp,
                            bias=cb_sb[:, lh, cls:cls + 1], scale=1.0,
                        )
                        srcs.append(pt)
                    for fn in (post_sched or {}).get(gi, []):
                        fn()
                    for _ in range(post_rate):
                        if filler:
                            filler.pop(0)()
                    if cls == 0:
                        for ah in range(2):
                            lh = 2 * hp + ah
                            pt = srcs[ah]
                            src = ppool.tile([128, 1024], f16, tag="src",
                                             bufs=8,
                                             name=f"src{u}{hp}{g}{ah}")
                            for ti in range(2):
                                col = _ebt_col(2 * g + ti, u)
                                nc.vector.tensor_mul(
                                    src[:, ti * 512:(ti + 1) * 512],
                                    pt[:, ti * 512:(ti + 1) * 512],
                                    ebt_sb[:, lh, col:col + 512],
                                )
                            srcs[ah] = src
                    avq.append((g, srcs))
                    if len(avq) > av_lag:
                        av_emit(*avq.pop(0))
                while avq:
                    av_emit(*avq.pop(0))
                return ctxps

            # ---- block (0,0): explicit schedules ---------------------------
            # gorder(0) = [4,5,6,7,3,2,1,0]
            pre00 = {
                4: [pg_item(wk_sb, k_rhs, kt_sb, 0, 1)],
                6: [pg_item(wk_sb, k_rhs, kt_sb, 0, 0)],
            }
            post00 = {
                2: [v_group(4)],
                3: [v_group(5), v_group(6)],
                4: [v_group(7), v_group(3)],
                5: [v_group(2),
                    pg_item(wk_sb, k_rhs, kt_sb, 1, 0),
                    pg_item(wk_sb, k_rhs, kt_sb, 1, 1)],
                6: [v_group(1),
                    pg_item(wk_sb, k_rhs, kt_sb, 1, 2),
                    pg_item(wk_sb, k_rhs, kt_sb, 1, 3)],
                7: [v_group(0)],
            }
            ctxps = attention(0, 0, pre00, post00, post_rate=0, av_lag=3)
            filler.extend([pg_item(wq_sb, q_rhs, qt_sb, 1, 1),
                           pg_item(wq_sb, q_rhs, qt_sb, 1, 2),
                           pg_item(wq_sb, q_rhs, qt_sb, 1, 3),
                           pg_item(wq_sb, q_rhs, qt_sb, 0, 1),
                           pg_item(wq_sb, q_rhs, qt_sb, 0, 2),
                           pg_item(wq_sb, q_rhs, qt_sb, 0, 3)])
            filler.extend(epilogue_items(0, 0, ctxps))

            # hp1 unit-0 Q projection must precede block (0,1)'s scores
            pre01 = {0: [pg_item(wq_sb, q_rhs, qt_sb, 1, 0)]}
            ctxps = attention(0, 1, pre01, None, post_rate=2)
            filler.extend(epilogue_items(0, 1, ctxps))
            filler.extend(oproj_items(0))

            for u in range(1, NU):
                for hp in range(2):
                    last = (u == NU - 1 and hp == 1)
                    ctxps = attention(u, hp, post_rate=2 if last else 1)
                    filler.extend(epilogue_items(u, hp, ctxps))
                    if hp == 1 and not last:
                        filler.extend(oproj_items(u))
                    if u == NU - 1 and hp == 0:
                        # stage the hp0-heads O-projection partials inside
                        # the last attention block
                        filler.extend(oproj_a_item(u, qs) for qs in range(4))
            # tail: keep the PE warm through the final normalization chain,
            # then finish the split O-projection
            tail = list(filler)
            filler.clear()
            for i, fn in enumerate(tail):
                fn()
                if i == 0:
                    keep_warm()
            for qs in range(4):
                oproj_b_item(NU - 1, qs)()

    nc.compile()
    return nc


_PROGRAM = None


def _get_program():
    global _PROGRAM
    if _PROGRAM is None:
        _PROGRAM = build_program()
    return _PROGRAM


def kernel(**inputs):
    query = np.asarray(inputs["query"], dtype=np.float32)
    key = np.asarray(inputs["key"], dtype=np.float32)
    value = np.asarray(inputs["value"], dtype=np.float32)
    mask = np.asarray(inputs["mask"])
    Wq = np.asarray(inputs["Wq"], dtype=np.float32)
    Wk = np.asarray(inputs["Wk"], dtype=np.float32)
    Wv = np.asarray(inputs["Wv"], dtype=np.float32)
    Wo = np.asarray(inputs["Wo"], dtype=np.float32)
    bo = np.asarray(inputs["bo"], dtype=np.float32)
    rel_bias = np.asarray(inputs["rel_bias"], dtype=np.float32)

    if not np.all(mask != 0):
        raise NotImplementedError("kernel assumes an all-ones attention mask")

    nc = _get_program()
    scale = np.float32(1.0 / np.sqrt(DK))

    # sliding-window exp-bias table: ebt[p, lh, j] = exp(tbl[clip(895-j+p)])
    pp = np.arange(128)[:, None]
    jj = np.arange(1408)[None, :]
    widx = np.clip(895 - jj + pp, 0, 510)  # [128, 1408]

    in_maps = []
    for c in range(NCORES):
        b = c // 2
        hbase = (c % 2) * 4
        rows = slice(hbase * 64, (hbase + 4) * 64)

        wq_arr = np.ascontiguousarray(
            (Wq[rows, :] * scale).T.reshape(4, 128, 256).swapaxes(0, 1))
        wk_arr = np.ascontiguousarray(
            Wk[rows, :].T.reshape(4, 128, 256).swapaxes(0, 1))
        wv_arr = np.ascontiguousarray(
            Wv[rows, :].T.reshape(4, 128, 256).swapaxes(0, 1))

        wo_arr = np.empty((64, 4, 512), dtype=np.float32)
        ebt_arr = np.empty((128, 4, 1408), dtype=np.float16)
        cb_arr = np.zeros((128, 4, 3), dtype=np.float32)
        for lh in range(4):
            g = hbase + lh
            wo_arr[:, lh, :] = Wo[:, g * 64:(g + 1) * 64].T * (1.0 / 256.0)
            tbl = rel_bias[g]
            ebt_arr[:, lh, :] = np.exp(tbl)[widx]
            cb_arr[:, lh, 1] = tbl[0]
            cb_arr[:, lh, 2] = tbl[510]

        bf = np.float16
        in_maps.append({
            "xqT": np.ascontiguousarray(query[b].T).astype(bf),
            "xkT": np.ascontiguousarray(key[b].T).astype(bf),
            "xvT": np.ascontiguousarray(value[b].T).astype(bf),
            "wq": wq_arr.astype(bf), "wk": wk_arr.astype(bf),
            "wv": wv_arr.astype(bf), "wo": wo_arr.astype(bf),
            "ebt": ebt_arr, "cb": cb_arr,
        })

    res = run_bass_kernel_spmd(nc, in_maps, list(range(NCORES)), trace=False)

    out = np.zeros((B, S, D), dtype=np.float32)
    for c in range(NCORES):
        out[c // 2] += res.results[c]["out"]
    out += bo[None, None, :]
    return out


# revision 13
# speedup vs baseline: 1.3514x; 1.0334x over previous
"""Trainium2 Bass kernel for nn_MultiHeadAttention_34162169872901.

MultiHeadAttention (B=4, S=2048, d_model=512, 8 heads, d_k=64) with a
relative-position bias table (511 entries, clamp +-255) and an all-ones mask.

Sharding (8 NeuronCores): core c handles batch b = c//2 and 4 of the 8 heads
(c%2 selects the head half) -- data parallel on B, tensor parallel on heads.
Each core computes its 4 heads' Q/K/V projections, the full attention for its
batch, normalization, and its partial output projection; the host sums the two
partial outputs per batch (and adds the output bias bo).

v4 -- scheduled for the ACT (exp) bottleneck:
  - The softmax exp on the scalar/ACT engine is the binding resource
    (128 x [128,1024] exps ~= 1.13us each).  The schedule starts exp as
    early as possible and tries to never let ACT starve.
  - DMA triggers cost ~650ns each and serialize per engine, so inputs are
    consolidated into a few multi-block tiles, triggered first thing in
    the program, in need order, on the two fast queues (sync + gpsimd).
  - Score matmuls are emitted ah-major so the quadrant whose PSUM bank
    frees first is refilled without waiting for the second exp.
  - AV matmuls are globally deferred by two groups (and carry across
    block boundaries) so filler never blocks the score stream and the
    next block's first scores precede the previous block's AV drain.
  - The relative-bias exp-table is a [128, 4, 1408] sliding-window table
    (every in-band Toeplitz block is a contiguous 512-col slice).
  - PSUM: scores ring 2x[128,1024] (4 banks), AV-accumulator ring
    2x[65,512] (2 banks), epilogue/projection ring 2x[128,512] (2 banks).
  - Per-(u,hp) normalization and the O-projection are pipelined into the
    following attention block; the final O-projection is split so half is
    staged during the last block, with keep-warm matmuls bridging the
    final normalization chain.
"""

import sys
import types

import numpy as np

B = 4
S = 2048
D = 512
NHEAD = 8
DK = 64
NCORES = 8
MAX_REL = 255
NKT = S // 128   # 16 k-tiles
NU = S // 512    # 4 q-units
NG = NKT // 2    # 8 score groups per (u, hp)


def _install_axon_hooks():
    """Provide antenv.axon_hooks (missing in this image) so bass_utils'
    trace path can be used; harmless when tracing is off."""
    try:
        import antenv
    except ImportError:
        return
    try:
        from antenv.axon_hooks import get_axon_ntff_profile_hook  # noqa: F401
        return
    except ImportError:
        pass
    hook = None
    try:
        from trn_agent_boot.trn_boot import _ntff_profile_via_ctypes
        hook = _ntff_profile_via_ctypes("/opt/axon/libaxon_pjrt.so")
    except Exception:
        hook = None
    m = types.ModuleType("antenv.axon_hooks")
    m.get_axon_ntff_profile_hook = lambda: hook
    m.set_axon_ntff_profile_hook = lambda h: None
    sys.modules["antenv.axon_hooks"] = m
    antenv.axon_hooks = m


_install_axon_hooks()

import concourse.bass as bass  # noqa: E402
import concourse.bacc as bacc  # noqa: E402
import concourse.mybir as mybir  # noqa: E402
from concourse import tile  # noqa: E402
from concourse.bass_utils import run_bass_kernel_spmd  # noqa: E402
from concourse.vector_clock import ScopedClock as _ScopedClock  # noqa: E402

f32 = mybir.dt.float32
bf16 = mybir.dt.bfloat16
f16 = mybir.dt.float16
AF = mybir.ActivationFunctionType


def _patched_drain_and_barrier(self, tick_clock, wait_clock):
    # walrus in this container rejects >2 sem waits on one instruction; emit
    # the tail-drain waits as standalone wait instructions instead.
    nc = self.nc
    dummy = mybir.InstNoOp(name="drain-wait-probe", engine=mybir.EngineType.SP)
    wait_clock.add_sem_waits(dummy, _ScopedClock({None: tick_clock.global_clock}))
    handles = {h.name: h for h in self.sems.allocated().values()}
    si = dummy.sync_info
    for w in (si.on_wait if si is not None else []):
        nc.sync.wait_ge(handles[w.ant_name], w.wait_value)
    nc.sync.drain()
    nc.all_engine_barrier()
    popped = nc._tile_sem_poison_stack.pop()
    assert popped is self._sem_poison
    nc.clear_and_free_semaphores(list(self.sems.allocated().values()))
    nc.all_engine_barrier()


tile.TileContext._drain_and_barrier = _patched_drain_and_barrier


def _delta(t, u):
    # key-tile offset minus query-chunk offset; bias entry index is
    # delta + (p - f) + 255 clipped to [0, 510]
    return 128 * t - 512 * u


def _cls(t, u):
    d = _delta(t, u)
    if d <= -384:
        return 1  # whole block clamps to table[0]
    if d >= 768:
        return 2  # whole block clamps to table[510]
    return 0      # in-band: needs the Toeplitz block


def _ebt_col(t, u):
    # start column of the [128,512] in-band block inside the 1408-wide
    # sliding-window exp-bias table: 640 - delta
    return 640 - _delta(t, u)


def _gorder(u):
    # out-of-band groups first (no bias-table dependency, constant folded
    # into the exp bias).  For u == 0, order by K-projection availability:
    # high k-columns first, in-band groups in descending g.
    if u == 0:
        return [4, 5, 6, 7, 3, 2, 1, 0]
    def key(g):
        return (_cls(2 * g, u) == 0, g)
    return sorted(range(NG), key=key)


def build_program():
    nc = bacc.Bacc()

    xqT = nc.declare_dram_parameter("xqT", [D, S], f16, isOutput=False)
    xkT = nc.declare_dram_parameter("xkT", [D, S], f16, isOutput=False)
    xvT = nc.declare_dram_parameter("xvT", [D, S], f16, isOutput=False)
    wq = nc.declare_dram_parameter("wq", [128, 4, 256], f16, isOutput=False)
    wk = nc.declare_dram_parameter("wk", [128, 4, 256], f16, isOutput=False)
    wv = nc.declare_dram_parameter("wv", [128, 4, 256], f16, isOutput=False)
    wo = nc.declare_dram_parameter("wo", [64, 4, 512], f16, isOutput=False)
    ebtd = nc.declare_dram_parameter("ebt", [128, 4, 1408], f16, isOutput=False)
    cbd = nc.declare_dram_parameter("cb", [128, 4, 3], f32, isOutput=False)
    outd = nc.declare_dram_parameter("out", [S, D], f32, isOutput=True)

    with tile.TileContext(nc) as tc:
        with (
            tc.tile_pool(name="sb", bufs=1) as pool,
            tc.tile_pool(name="xt", bufs=1) as xpool,
            tc.tile_pool(name="pt", bufs=14) as ppool,
            tc.tile_pool(name="cxp", bufs=2) as cpool,
            tc.tile_pool(name="obp", bufs=4) as opool,
            tc.tile_pool(name="sc", bufs=2, space="PSUM") as scp,
            tc.tile_pool(name="cx", bufs=2, space="PSUM") as ctxpool,
            tc.tile_pool(name="ep", bufs=2, space="PSUM") as epp,
        ):
            # ---- persistent SBUF tiles -------------------------------------
            wq_sb = pool.tile([128, 4, 256], f16, tag="wq")
            wk_sb = pool.tile([128, 4, 256], f16, tag="wk")
            wv_sb = pool.tile([128, 4, 256], f16, tag="wv")
            wo_sb = pool.tile([64, 4, 512], f16, tag="wo")
            ebt_sb = pool.tile([128, 4, 1408], f16, tag="ebt")
            cb_sb = pool.tile([128, 4, 3], f32, tag="cb")
            qt_sb = pool.tile([128, 2, S], f16, tag="qt")
            kt_sb = pool.tile([128, 2, S], f16, tag="kt")
            v_sb = pool.tile([128, NKT, 4 * 65], f16, tag="v")
            ones_c = pool.tile([1, 64], f16, tag="ones")
            warm = pool.tile([128, 16], f32, tag="warm")

            # input tiles, consolidated so each is one DMA trigger:
            # xq0: q-unit 0 for all 4 d-blocks; xq123: q-units 1..3;
            # xka: k cols 1024:2048; xkbc: k cols 0:1024; xv01/xv23: V halves
            xq0 = xpool.tile([128, 4, 512], f16, tag="xq0")
            xq123 = xpool.tile([128, 4, 1536], f16, tag="xq123")
            xka = xpool.tile([128, 4, 1024], f16, tag="xka")
            xkbc = xpool.tile([128, 4, 1024], f16, tag="xkbc")
            xv01 = xpool.tile([128, 2, 2048], f16, tag="xv01")
            xv23 = xpool.tile([128, 2, 2048], f16, tag="xv23")

            def blk(t, cols):
                return t[:, cols].rearrange("(c p) s -> p c s", p=128)

            # ---- DMA triggers first (each costs ~650ns of engine time and
            # the data cannot start moving until its trigger runs) ----------
            nc.sync.dma_start(wq_sb[:], wq[:])
            nc.sync.dma_start(wv_sb[:], wv[:])
            nc.sync.dma_start(xq0[:], blk(xqT, slice(0, 512)))
            nc.sync.dma_start(xv01[:],
                              xvT[0:256, :].rearrange("(c p) s -> p c s", p=128))
            nc.sync.dma_start(xkbc[:], blk(xkT, slice(0, 1024)))
            nc.sync.dma_start(xq123[:], blk(xqT, slice(512, 2048)))
            nc.sync.dma_start(wo_sb[:], wo[:])
            nc.gpsimd.dma_start(wk_sb[:], wk[:])
            nc.gpsimd.dma_start(cb_sb[:], cbd[:])
            nc.gpsimd.dma_start(xka[:], blk(xkT, slice(1024, 2048)))
            nc.gpsimd.dma_start(xv23[:],
                                xvT[256:512, :].rearrange("(c p) s -> p c s",
                                                          p=128))
            nc.gpsimd.dma_start(ebt_sb[:], ebtd[:])

            # load the exp table set (one-time ~2.7us) while DMAs stream
            nc.vector.memset(warm[:], 0.0)
            nc.scalar.activation(warm[:], warm[:], AF.Exp, bias=0.0, scale=1.0)
            nc.vector.memset(ones_c[:], 1.0)

            def xv_slice(ct, cols):
                t = (xv01, xv23)[ct // 2]
                return t[:, ct % 2, cols]

            # ---- projection helpers ----------------------------------------
            def k_rhs(sc, ct):
                if sc < 2:
                    return xkbc[:, ct, sc * 512:(sc + 1) * 512]
                return xka[:, ct, (sc - 2) * 512:(sc - 1) * 512]

            def q_rhs(sc, ct):
                if sc == 0:
                    return xq0[:, ct, :]
                return xq123[:, ct, (sc - 1) * 512:sc * 512]

            def proj_group(w_sb, rhs_fn, dst, hp, sc, ptag):
                pk = (ctxpool if ptag == "cx" else epp).tile(
                    [128, 512], f32, tag=ptag, name=f"pj{hp}{sc}")
                for ct in range(4):
                    nc.tensor.matmul(
                        pk[:],
                        lhsT=w_sb[:, ct, hp * 128:(hp + 1) * 128],
                        rhs=rhs_fn(sc, ct),
                        start=(ct == 0), stop=(ct == 3),
                    )
                nc.vector.tensor_copy(dst[:, hp, sc * 512:(sc + 1) * 512], pk[:])

            # pre-loop projections: Q hp0 unit-0 and K hp0 high columns
            proj_group(wq_sb, q_rhs, qt_sb, 0, 0, "ep")
            proj_group(wk_sb, k_rhs, kt_sb, 0, 2, "cx")
            proj_group(wk_sb, k_rhs, kt_sb, 0, 3, "cx")

            def v_group(g):
                def emit():
                    pv = epp.tile([128, 512], f32, tag="ep", name=f"pv{g}")
                    for sti in range(2):
                        st = 2 * g + sti
                        for ct in range(4):
                            nc.tensor.matmul(
                                pv[:, sti * 256:sti * 256 + 256],
                                lhsT=xv_slice(ct, slice(st * 128, (st + 1) * 128)),
                                rhs=wv_sb[:, ct, :],
                                start=(ct == 0), stop=(ct == 3),
                            )
                    for sti in range(2):
                        st = 2 * g + sti
                        vslice = v_sb[:, st, :].rearrange(
                            "p (h x) -> p h x", x=65)
                        nc.vector.tensor_copy(
                            vslice[:, :, 0:64],
                            pv[:, sti * 256:sti * 256 + 256].rearrange(
                                "p (h x) -> p h x", x=64),
                        )
                        nc.vector.memset(vslice[:, :, 64:65], 1.0)
                return emit

            def pg_item(w_sb, rhs_fn, dst, hp, sc):
                def emit():
                    proj_group(w_sb, rhs_fn, dst, hp, sc, "ep")
                return emit

            def noop():
                pass

            # ---- attention + pipelined epilogue ----------------------------
            cx_tiles = {}     # (u, lh) -> normalized ctx [64, 512] f16
            ob_a = {}         # qs -> staged hp0 O-projection partial (f32)
            filler = []       # FIFO of emission closures
            avq = []          # globally deferred AV matmul emissions
            AV_LAG = 2

            def av_drain(n=None):
                k = len(avq) if n is None else n
                for _ in range(k):
                    if avq:
                        avq.pop(0)()

            def epilogue_items(u, hp, ctxps):
                """Normalization for the two heads of (u, hp), split into
                pipeline-friendly chunks (with no-op spacers so the serial
                chain never blocks an engine queue)."""
                state = {}

                def e1():
                    for ah in range(2):
                        ctxf = cpool.tile([65, 512], f32, tag="ctxf", bufs=4,
                                          name=f"ctxf{u}{hp}{ah}")
                        nc.vector.tensor_copy(ctxf[:], ctxps[ah][:])
                        state[ah] = ctxf

                def e2a():
                    lp = cpool.tile([1, 1024], f32, tag="lp", name=f"lp{u}{hp}")
                    nc.gpsimd.dma_start(lp[:, 0:512], state[0][64:65, :])
                    nc.gpsimd.dma_start(lp[:, 512:1024], state[1][64:65, :])
                    state["lp"] = lp

                def e2b():
                    linv = cpool.tile([1, 1024], f32, tag="linv",
                                      name=f"linv{u}{hp}")
                    nc.vector.reciprocal_approx_fast(linv[:], state["lp"][:])
                    linvb = cpool.tile([1, 1024], f16, tag="linvb",
                                       name=f"linvb{u}{hp}")
                    nc.vector.tensor_scalar_mul(linvb[:], linv[:], 256.0)
                    state["linvb"] = linvb

                def e3():
                    linvb = state["linvb"]
                    for ah in range(2):
                        bc = epp.tile([64, 512], f32, tag="ep",
                                      name=f"bc{u}{hp}{ah}")
                        nc.tensor.matmul(
                            bc[:], lhsT=ones_c[:],
                            rhs=linvb[:, ah * 512:(ah + 1) * 512],
                            start=True, stop=True)
                        cxn = cpool.tile([64, 512], f16, tag="cxn", bufs=8,
                                         name=f"cx{u}{hp}{ah}")
                        nc.vector.tensor_mul(cxn[:], bc[:], state[ah][0:64, :])
                        cx_tiles[(u, 2 * hp + ah)] = cxn

                return [e1, e2a, noop, e2b, noop, e3]

            def oproj_items(u):
                items = []
                for qs in range(4):
                    def emit(u=u, qs=qs):
                        po = epp.tile([128, 512], f32, tag="ep",
                                      name=f"po{u}{qs}")
                        for lh in range(4):
                            nc.tensor.matmul(
                                po[:],
                                lhsT=cx_tiles[(u, lh)][:, qs * 128:(qs + 1) * 128],
                                rhs=wo_sb[:, lh, :],
                                start=(lh == 0), stop=(lh == 3),
                            )
                        ob = opool.tile([128, 512], f32, tag="ob",
                                        name=f"ob{u}{qs}")
                        nc.vector.tensor_copy(ob[:], po[:])
                        nc.sync.dma_start(
                            outd[u * 512 + qs * 128: u * 512 + (qs + 1) * 128, :],
                            ob[:],
                        )
                    items.append(emit)
                return items

            # split O-projection for the last u: stage the hp0-heads partial
            # during the last attention block, finish + combine in the tail
            def oproj_a_item(u, qs):
                def emit():
                    po = epp.tile([128, 512], f32, tag="ep",
                                  name=f"poa{u}{qs}")
                    for lh in range(2):
                        nc.tensor.matmul(
                            po[:],
                            lhsT=cx_tiles[(u, lh)][:, qs * 128:(qs + 1) * 128],
                            rhs=wo_sb[:, lh, :],
                            start=(lh == 0), stop=(lh == 1),
                        )
                    oba = opool.tile([128, 512], f32, tag="oba", bufs=4,
                                     name=f"oba{u}{qs}")
                    nc.vector.tensor_copy(oba[:], po[:])
                    ob_a[qs] = oba
                return emit

            def oproj_b_item(u, qs):
                def emit():
                    po = epp.tile([128, 512], f32, tag="ep",
                                  name=f"pob{u}{qs}")
                    for lh in range(2, 4):
                        nc.tensor.matmul(
                            po[:],
                            lhsT=cx_tiles[(u, lh)][:, qs * 128:(qs + 1) * 128],
                            rhs=wo_sb[:, lh, :],
                            start=(lh == 2), stop=(lh == 3),
                        )
                    ob = opool.tile([128, 512], f32, tag="ob",
                                    name=f"ob{u}{qs}")
                    nc.vector.tensor_add(ob[:], po[:], ob_a[qs][:])
                    nc.sync.dma_start(
                        outd[u * 512 + qs * 128: u * 512 + (qs + 1) * 128, :],
                        ob[:],
                    )
                return emit

            def keep_warm(n=8):
                # junk matmuls bridging the tail normalization chain so HAM
                # doesn't re-throttle the PE before the final O-projection
                dz = ctxpool.tile([65, 512], f32, tag="cx", name="dz")
                for r in range(n):
                    vsl = v_sb[:, r, :].rearrange(
                        "p (h x) -> p h x", x=65)[:, 0, :]
                    nc.tensor.matmul(dz[:], lhsT=vsl, rhs=qt_sb[:, 0, 0:512],
                                     start=True, stop=True)

            def attention(u, hp, pre_sched=None, post_sched=None, post_rate=1):
                """pre_sched/post_sched: {gi: [closures]} emitted before the
                scores (pre) or between the exps and the AV matmuls (post) of
                group gi.  post_rate: queued filler items popped at each post
                point.  AV matmuls are appended to the global deferred queue
                and drained AV_LAG groups later (carrying across blocks)."""
                ctxps = [
                    ctxpool.tile([65, 512], f32, tag="cx",
                                 name=f"ctxp{u}{hp}{i}")
                    for i in range(2)
                ]
                nav = [0, 0]

                def av_item(g, srcs):
                    def emit():
                        for ah in range(2):
                            lh = 2 * hp + ah
                            for ti in range(2):
                                t = 2 * g + ti
                                vsl = v_sb[:, t, :].rearrange(
                                    "p (h x) -> p h x", x=65)[:, lh, :]
                                nav[ah] += 1
                                nc.tensor.matmul(
                                    ctxps[ah][:],
                                    lhsT=vsl,
                                    rhs=srcs[ah][:, ti * 512:(ti + 1) * 512],
                                    start=(nav[ah] == 1),
                                    stop=(nav[ah] == NKT),
                                )
                    return emit

                for gi, g in enumerate(_gorder(u)):
                    for fn in (pre_sched or {}).get(gi, []):
                        fn()
                    cls = _cls(2 * g, u)
                    sct = [scp.tile([128, 1024], f32, tag="sc",
                                    name=f"sct{u}{hp}{g}{i}")
                           for i in range(2)]
                    # ah-major: refill the freed PSUM bank's quadrant first
                    for ah in range(2):
                        for ti in range(2):
                            t = 2 * g + ti
                            nc.tensor.matmul(
                                sct[ah][:, ti * 512:(ti + 1) * 512],
                                lhsT=kt_sb[ah * 64:(ah + 1) * 64, hp,
                                           t * 128:(t + 1) * 128],
                                rhs=qt_sb[ah * 64:(ah + 1) * 64, hp,
                                          u * 512:(u + 1) * 512],
                                start=True, stop=True,
                                tile_position=(ah * 64, 0),
                            )
                    srcs = []
                    for ah in range(2):
                        lh = 2 * hp + ah
                        pt = ppool.tile([128, 1024], f16, tag="pt",
                                        name=f"pt{u}{hp}{g}{ah}")
                        nc.scalar.activation(
                            pt[:], sct[ah][:], AF.Exp,
                            bias=cb_sb[:, lh, cls:cls + 1], scale=1.0,
                        )
                        srcs.append(pt)
                    if gi == 0:
                        # the previous block's carried AV matmuls must all be
                        # emitted before its epilogue (a filler item below)
                        # can be: dependency tracking only covers emitted
                        # writers
                        av_drain()
                    for fn in (post_sched or {}).get(gi, []):
                        fn()
                    for _ in range(post_rate):
                        if filler:
                            filler.pop(0)()
                    if cls == 0:
                        for ah in range(2):
                            lh = 2 * hp + ah
                            pt = srcs[ah]
                            src = ppool.tile([128, 1024], f16, tag="src",
                                             bufs=8,
                                             name=f"src{u}{hp}{g}{ah}")
                            for ti in range(2):
                                col = _ebt_col(2 * g + ti, u)
                                nc.vector.tensor_mul(
                                    src[:, ti * 512:(ti + 1) * 512],
                                    pt[:, ti * 512:(ti + 1) * 512],
                                    ebt_sb[:, lh, col:col + 512],
                                )
                            srcs[ah] = src
                    avq.append(av_item(g, srcs))
                    while len(avq) > AV_LAG:
                        avq.pop(0)()
                return ctxps

            # ---- block (0,0): explicit schedules ---------------------------
            # gorder(0) = [4,5,6,7,3,2,1,0]
            pre00 = {
                4: [pg_item(wk_sb, k_rhs, kt_sb, 0, 1)],
                6: [pg_item(wk_sb, k_rhs, kt_sb, 0, 0)],
            }
            post00 = {
                1: [v_group(4)],
                2: [v_group(5), v_group(6)],
                3: [v_group(7), v_group(3)],
                4: [v_group(2)],
                5: [v_group(1),
                    pg_item(wk_sb, k_rhs, kt_sb, 1, 0),
                    pg_item(wk_sb, k_rhs, kt_sb, 1, 1)],
                6: [v_group(0),
                    pg_item(wk_sb, k_rhs, kt_sb, 1, 2),
                    pg_item(wk_sb, k_rhs, kt_sb, 1, 3)],
            }
            ctxps = attention(0, 0, pre00, post00, post_rate=0)
            filler.extend([pg_item(wq_sb, q_rhs, qt_sb, 1, 1),
                           pg_item(wq_sb, q_rhs, qt_sb, 1, 2),
                           pg_item(wq_sb, q_rhs, qt_sb, 1, 3),
                           pg_item(wq_sb, q_rhs, qt_sb, 0, 1),
                           pg_item(wq_sb, q_rhs, qt_sb, 0, 2),
                           pg_item(wq_sb, q_rhs, qt_sb, 0, 3)])
            filler.extend(epilogue_items(0, 0, ctxps))

            # hp1 unit-0 Q projection must precede block (0,1)'s scores
            pre01 = {0: [pg_item(wq_sb, q_rhs, qt_sb, 1, 0)]}
            ctxps = attention(0, 1, pre01, None, post_rate=2)
            filler.extend(epilogue_items(0, 1, ctxps))
            filler.extend(oproj_items(0))

            for u in range(1, NU):
                for hp in range(2):
                    last = (u == NU - 1 and hp == 1)
                    ctxps = attention(u, hp, post_rate=2 if last else 1)
                    filler.extend(epilogue_items(u, hp, ctxps))
                    if hp == 1 and not last:
                        filler.extend(oproj_items(u))
                    if u == NU - 1 and hp == 0:
                        # stage the hp0-heads O-projection partials inside
                        # the last attention block
                        filler.extend(oproj_a_item(u, qs) for qs in range(4))
            # tail: drain deferred AVs, keep the PE warm through the final
            # normalization chain, then finish the split O-projection
            av_drain()
            tail = list(filler)
            filler.clear()
            for i, fn in enumerate(tail):
                fn()
                if i in (0, 2, 4):
                    keep_warm(8)
            for qs in range(4):
                oproj_b_item(NU - 1, qs)()

    nc.compile()
    return nc


_PROGRAM = None


def _get_program():
    global _PROGRAM
    if _PROGRAM is None:
        _PROGRAM = build_program()
    return _PROGRAM


def kernel(**inputs):
    query = np.asarray(inputs["query"], dtype=np.float32)
    key = np.asarray(inputs["key"], dtype=np.float32)
    value = np.asarray(inputs["value"], dtype=np.float32)
    mask = np.asarray(inputs["mask"])
    Wq = np.asarray(inputs["Wq"], dtype=np.float32)
    Wk = np.asarray(inputs["Wk"], dtype=np.float32)
    Wv = np.asarray(inputs["Wv"], dtype=np.float32)
    Wo = np.asarray(inputs["Wo"], dtype=np.float32)
    bo = np.asarray(inputs["bo"], dtype=np.float32)
    rel_bias = np.asarray(inputs["rel_bias"], dtype=np.float32)

    if not np.all(mask != 0):
        raise NotImplementedError("kernel assumes an all-ones attention mask")

    nc = _get_program()
    scale = np.float32(1.0 / np.sqrt(DK))

    # sliding-window exp-bias table: ebt[p, lh, j] = exp(tbl[clip(895-j+p)])
    pp = np.arange(128)[:, None]
    jj = np.arange(1408)[None, :]
    widx = np.clip(895 - jj + pp, 0, 510)  # [128, 1408]

    in_maps = []
    for c in range(NCORES):
        b = c // 2
        hbase = (c % 2) * 4
        rows = slice(hbase * 64, (hbase + 4) * 64)

        wq_arr = np.ascontiguousarray(
            (Wq[rows, :] * scale).T.reshape(4, 128, 256).swapaxes(0, 1))
        wk_arr = np.ascontiguousarray(
            Wk[rows, :].T.reshape(4, 128, 256).swapaxes(0, 1))
        wv_arr = np.ascontiguousarray(
            Wv[rows, :].T.reshape(4, 128, 256).swapaxes(0, 1))

        wo_arr = np.empty((64, 4, 512), dtype=np.float32)
        ebt_arr = np.empty((128, 4, 1408), dtype=np.float16)
        cb_arr = np.zeros((128, 4, 3), dtype=np.float32)
        for lh in range(4):
            g = hbase + lh
            wo_arr[:, lh, :] = Wo[:, g * 64:(g + 1) * 64].T * (1.0 / 256.0)
            tbl = rel_bias[g]
            ebt_arr[:, lh, :] = np.exp(tbl)[widx]
            cb_arr[:, lh, 1] = tbl[0]
            cb_arr[:, lh, 2] = tbl[510]

        bf = np.float16
        in_maps.append({
            "xqT": np.ascontiguousarray(query[b].T).astype(bf),
            "xkT": np.ascontiguousarray(key[b].T).astype(bf),
            "xvT": np.ascontiguousarray(value[b].T).astype(bf),
            "wq": wq_arr.astype(bf), "wk": wk_arr.astype(bf),
            "wv": wv_arr.astype(bf), "wo": wo_arr.astype(bf),
            "ebt": ebt_arr, "cb": cb_arr,
        })

    res = run_bass_kernel_spmd(nc, in_maps, list(range(NCORES)), trace=False)

    out = np.zeros((B, S, D), dtype=np.float32)
    for c in range(NCORES):
        out[c // 2] += res.results[c]["out"]
    out += bo[None, None, :]
    return out
